# revision 8
# baseline (speedup 1.0000x reference)
"""DeepseekV3 FP8-block-dequant SwiGLU MLP on 8 TRN2 NeuronCores.

Computation: y = (silu(x @ dq(w_gate).T) * (x @ dq(w_up).T)) @ dq(w_down).T
with dq(w)[o,i] = w[o,i] * s[o//128, i//128].

Sharding: tensor-parallel over the F=2048 intermediate dim, 256 per core.
Each core computes a rank-256 partial of the output; partials are summed
on the host (the unshard step for a contraction-sharded output).

Device layout (prepared host-side, bf16):
  xp   [128, 56*512] : xp[p, k*512+t] = x[t, k*128+p]            (same on all cores)
  wgu  [128, 56*512] : wgu[p, k*512+m*128+f] = w_gate[c*256+m*128+f, k*128+p]; +256 up
  wdp  [128, 2*7168] : wdp[p, k2*7168+h] = w_down[h, c*256+k2*128+p]
  sgb  [128, 336]    : bf16 scale grid rows, 56 cols per block-row:
                       [gate m=0 | gate m=1 | up m=0 | up m=1 | down k2=0 | down k2=1]
  sgu  [128, 224]    : tiny fp32 grid, only used as PE-warmup matmul fodder
All matmuls contract over the partition dim. Dequant is elementwise
w *= scale done in-place in the weight landing buffers by wide DVE
tensor-tensor ops whose scale operand is a stride-0 broadcast AP over the
tiny sgb grid (each 128-column group of a k-tile shares one scale value),
so no full-size scale tensor is ever DMAed: per-core HBM traffic is
25.7 MB vs 36.7 MB with materialized scale broadcasts. At the ~340 GB/s
per-core DMA ceiling that is the difference between 108 us (DMA-bound)
and ~76 us (near the 71.7 us bf16 PE roofline).
"""

import sys

if "/opt/trn_rl_repo" not in sys.path:
    sys.path.insert(0, "/opt/trn_rl_repo")

from contextlib import ExitStack

import ml_dtypes
import numpy as np

import concourse.bacc as bacc
import concourse.mybir as mybir
import concourse.tile as tile
from concourse import bass_utils
from concourse.bass import AP as _AP

T, H, F = 512, 7168, 2048
NCORES = 8
FC = F // NCORES  # 256 intermediate channels per core
KT = H // 128  # 56 contraction k-tiles for gate/up
HN = H // 512  # 14 output column chunks for down matmul
BF16 = mybir.dt.bfloat16
F32 = mybir.dt.float32

_CACHE = {}


def _bview(ap, dims, off=0):
    """AP over `ap`'s tensor: keep the partition dim, replace the free dims
    with explicit (stride, size) pairs (stride 0 = broadcast), shift by off
    elements."""
    return _AP(ap.tensor, ap.offset + off, [list(ap.ap[0])] + [[s, n] for s, n in dims])


def _build_program(repeats=1, warmup=True):
    nc = bacc.Bacc("TRN2", target_bir_lowering=False, debug=False, num_devices=NCORES)

    xd = nc.dram_tensor("xp", [128, KT * T], BF16, kind="ExternalInput")
    wgud = nc.dram_tensor("wgu", [128, KT * 2 * FC], BF16, kind="ExternalInput")
    wdd = nc.dram_tensor("wdp", [128, 2 * H], BF16, kind="ExternalInput")
    sgbd = nc.dram_tensor("sgb", [128, 6 * KT], BF16, kind="ExternalInput")
    sgud = nc.dram_tensor("sgu", [128, 4 * KT], F32, kind="ExternalInput")
    yd = nc.dram_tensor("y", [T, H], BF16, kind="ExternalOutput")

    with tile.TileContext(nc) as tc, ExitStack() as ctx:
        consts = ctx.enter_context(tc.tile_pool(name="consts", bufs=1))
        xpool = ctx.enter_context(tc.tile_pool(name="xpool", bufs=3))
        wpool = ctx.enter_context(tc.tile_pool(name="wpool", bufs=3))
        silpool = ctx.enter_context(tc.tile_pool(name="silpool", bufs=2))
        hpool = ctx.enter_context(tc.tile_pool(name="hpool", bufs=2))
        wdraw_pool = ctx.enter_context(tc.tile_pool(name="wdraw", bufs=2))
        ystage = ctx.enter_context(tc.tile_pool(name="ystage", bufs=2))
        pgu = ctx.enter_context(tc.tile_pool(name="pgu", bufs=4, space="PSUM"))
        pd = ctx.enter_context(tc.tile_pool(name="pd", bufs=4, space="PSUM"))

        sgu_sb = consts.tile([128, 4 * KT], F32, name="sgu_sb", tag="sgu_sb")
        nc.sync.dma_start(sgu_sb[:], sgud.ap())
        sgb_sb = consts.tile([128, 6 * KT], BF16, name="sgb_sb", tag="sgb_sb")
        nc.sync.dma_start(sgb_sb[:], sgbd.ap())
        # dummy sigmoid at kernel start: loads the ACT sigmoid LUT while the
        # pipeline fills, so the real sigmoids at the phase-1->2 transition
        # don't pay the ~1.3us table-load on the critical path
        sig_warm = consts.tile([128, 1], BF16, name="sig_warm", tag="sig_warm")
        nc.scalar.activation(
            sig_warm[:], sgu_sb[:, :1], mybir.ActivationFunctionType.Sigmoid
        )

        # ---- PE warmup (once per program, NOT per repeat body): dummy fp32
        # matmuls on the (tiny, already-loaded) scale grid keep the PE HAM
        # activity window busy during the DMA pipeline fill, so the real
        # matmul stream starts at 2.4 GHz. In a repeat stream the PE stays
        # hot, so later bodies don't need (or pay for) the warmup.
        ps_warm = pd.tile([128, 512], F32, name="ps_warm", tag="pd")
        for _ in range(10 if warmup else 0):
            nc.tensor.matmul(
                ps_warm[:, : 2 * KT],
                sgu_sb[:, :128],
                sgu_sb[:, : 2 * KT],
                start=True,
                stop=True,
            )

        def dq_gu(wc, k0, j0, cs):
            """Dequant k-tiles [j0, j0+cs) of the chunk starting at k-tile k0:
            wc cols (k, group g of 4, 128 inner) *= sgb[g*KT + k0+k],
            broadcast over the 128 inner columns."""
            wv = _bview(wc[:], [(512, cs), (128, 4), (1, 128)], off=j0 * 512)
            sv = _bview(sgb_sb[:], [(1, cs), (KT, 4), (0, 128)], off=k0 + j0)
            nc.vector.tensor_mul(wv, wv, sv)

        def dq_d(wdr, k2, n):
            """Dequant down-weight cols [n*512,(n+1)*512) of the k2 tile:
            4 hb groups of 128 *= sgb[4*KT + k2*KT + 4n + hb]."""
            wv = _bview(wdr[:], [(128, 4), (1, 128)], off=n * 512)
            sv = _bview(sgb_sb[:], [(1, 4), (0, 128)], off=4 * KT + k2 * KT + 4 * n)
            nc.vector.tensor_mul(wv, wv, sv)

        def emit_body():
            # ---- phase 1: gT/uT = dq(w).T @ x.T tiles, accumulated over 56 k
            psg = [pgu.tile([128, T], F32, name=f"psg{m}", tag="p1") for m in range(2)]
            psu = [pgu.tile([128, T], F32, name=f"psu{m}", tag="p1") for m in range(2)]

            # moderate first chunk (fewer chunk boundaries beat tiny head
            # chunks); small last chunk keeps the post-DMA compute tail short
            chunks = [4, 8, 8, 8, 8, 8, 8, 4]
            assert sum(chunks) == KT

            k = 0
            for g, cs in enumerate(chunks):
                nb = {4: 2, 8: 3}[cs]
                cols = slice(k * T, (k + cs) * T)
                wc = wpool.tile(
                    [128, cs * 2 * FC], BF16, name=f"wc{g}", tag=f"wc{cs}", bufs=nb
                )
                nc.sync.dma_start(wc[:], wgud.ap()[:, cols])
                xc = xpool.tile(
                    [128, cs * T], BF16, name=f"xc{g}", tag=f"xc{cs}", bufs=nb
                )
                nc.sync.dma_start(xc[:], xd.ap()[:, cols])
                # in-place dequant of the chunk; the last (small) chunk is
                # dequantized in two halves so its matmuls are not gated on
                # one full-chunk DVE op right at the phase-1 tail
                if g == len(chunks) - 1 and cs >= 2:
                    dq_gu(wc, k, 0, cs // 2)
                    dq_gu(wc, k, cs // 2, cs - cs // 2)
                else:
                    dq_gu(wc, k, 0, cs)
                for j in range(cs):
                    start, stop = (k == 0), (k == KT - 1)
                    rhs = xc[:, j * T : (j + 1) * T]
                    for m in range(2):
                        nc.tensor.matmul(
                            psg[m][:],
                            wc[:, j * 512 + m * 128 : j * 512 + (m + 1) * 128],
                            rhs,
                            start=start,
                            stop=stop,
                        )
                        nc.tensor.matmul(
                            psu[m][:],
                            wc[:, j * 512 + 256 + m * 128 : j * 512 + (m + 1) * 128 + 256],
                            rhs,
                            start=start,
                            stop=stop,
                        )
                    k += 1

            # ---- down-proj weight loads on SP's queue after the gate/up
            # chunk reads. SP's per-body stream (18.4 MB, ~54 us) drains
            # faster than the body period, so with cross-body pipelining
            # these arrive well before phase 2 needs them (~48 us in); the
            # ACT queue carries only y writes and must NOT carry these (ACT
            # DMA issues are in-order with ACT compute, which would delay
            # them behind the previous body's PSUM-drain copies).
            wdr = [
                wdraw_pool.tile([128, H], BF16, name=f"wdr{i}", tag="wdr")
                for i in range(2)
            ]
            for half in range(2):
                lo, hi = half * (H // 2), (half + 1) * (H // 2)
                for k2 in range(2):
                    nc.sync.dma_start(
                        wdr[k2][:, lo:hi], wdd.ap()[:, k2 * H + lo : k2 * H + hi]
                    )

            # dequant only the FIRST phase-2 weight chunk before the h chain:
            # the first down-matmul needs h(t=0) and wdq(n=0) — both on DVE's
            # in-order queue — so anything more here just delays the h chain
            for k2 in range(2):
                dq_d(wdr[k2], k2, 0)

            # ---- h = silu(g) * u = sigmoid(g) * g * u, in [128, 128] column
            # slices so phase 2's t=0 matmuls can start early
            sil = [
                silpool.tile([128, T], BF16, name=f"sil{m}", tag="sil")
                for m in range(2)
            ]
            tmp = [
                silpool.tile([128, T], BF16, name=f"sgm{m}", tag="sgm")
                for m in range(2)
            ]
            hts = [
                hpool.tile([128, T], BF16, name=f"ht{m}", tag="ht") for m in range(2)
            ]
            for tt in range(4):
                sl = slice(tt * 128, (tt + 1) * 128)
                for m in range(2):
                    # sigmoid + two muls rather than Silu: CoreSim lacks Silu,
                    # and on HW the Silu LUT measured slower end-to-end
                    nc.scalar.activation(
                        sil[m][:, sl],
                        psg[m][:, sl],
                        mybir.ActivationFunctionType.Sigmoid,
                    )
                    nc.vector.tensor_mul(tmp[m][:, sl], sil[m][:, sl], psg[m][:, sl])
                    nc.vector.tensor_mul(hts[m][:, sl], tmp[m][:, sl], psu[m][:, sl])

            # ---- remaining phase 2 in-place dequant (n-major so early n
            # chunks are ready first), then y_partial[t, h] = hT.T @ dq(wd)
            for n in range(1, HN):
                for k2 in range(2):
                    dq_d(wdr[k2], k2, n)

            for t in range(4):
                for half in range(2):
                    ystg = ystage.tile([128, H // 2], BF16, name=f"ys{t}{half}", tag="ys")
                    for nh in range(HN // 2):
                        n = half * (HN // 2) + nh
                        ps = pd.tile([128, 512], F32, name=f"ps{t}_{n}", tag="pd")
                        for k2 in range(2):
                            nc.tensor.matmul(
                                ps[:],
                                hts[k2][:, t * 128 : (t + 1) * 128],
                                wdr[k2][:, n * 512 : (n + 1) * 512],
                                start=(k2 == 0),
                                stop=(k2 == 1),
                            )
                        dst = ystg[:, nh * 512 : (nh + 1) * 512]
                        if (n + t) % 4 == 0:
                            nc.vector.tensor_copy(dst, ps[:])
                        else:
                            nc.scalar.copy(dst, ps[:])
                        # split the very last output transfer so the DMA tail
                        # after the final copy is short. y writes go on the
                        # ACT DMA queue: they'd otherwise sit in SP's queue
                        # ahead of the NEXT body's weight reads and stall
                        # them whenever phase-2 compute is still producing y.
                        if t == 3 and half == 1 and nh == 3:
                            nc.scalar.dma_start(
                                yd.ap()[
                                    t * 128 : (t + 1) * 128,
                                    H // 2 : H // 2 + 4 * 512,
                                ],
                                ystg[:, : 4 * 512],
                            )
                    lo = 4 * 512 if (t == 3 and half == 1) else 0
                    nc.scalar.dma_start(
                        yd.ap()[
                            t * 128 : (t + 1) * 128,
                            half * (H // 2) + lo : (half + 1) * (H // 2),
                        ],
                        ystg[:, lo:],
                    )

        for _rep in range(repeats):
            emit_body()

    nc.compile()
    return nc


def _get_program():
    if "nc" not in _CACHE:
        _CACHE["nc"] = _build_program()
    return _CACHE["nc"]


def _prep_inputs(x, w_gate, s_gate, w_up, s_up, w_down, s_down):
    bf = ml_dtypes.bfloat16
    # x -> [p, k, t] -> [128, KT*T]
    xp = np.ascontiguousarray(
        x.reshape(T, KT, 128).transpose(2, 1, 0).reshape(128, KT * T)
    ).astype(bf)
    in_maps = []
    for c in range(NCORES):
        gsl = slice(c * FC, (c + 1) * FC)
        ag = w_gate[gsl].reshape(FC, KT, 128).transpose(2, 1, 0)  # [p, k, f]
        au = w_up[gsl].reshape(FC, KT, 128).transpose(2, 1, 0)
        wgu = np.ascontiguousarray(
            np.concatenate([ag, au], axis=2).reshape(128, KT * 2 * FC)
        ).astype(bf)
        wdp = np.ascontiguousarray(
            w_down[:, gsl].reshape(H, 2, 128).transpose(2, 1, 0).reshape(128, 2 * H)
        ).astype(bf)
        # scale grid rows (bf16, replication only): 56 cols per block-row of
        # this core's weights, in the order the broadcast-AP dequant reads:
        # [gate m=0 | gate m=1 | up m=0 | up m=1 | down k2=0 | down k2=1]
        srow = np.concatenate(
            [
                s_gate[2 * c],
                s_gate[2 * c + 1],
                s_up[2 * c],
                s_up[2 * c + 1],
                s_down[:, 2 * c],
                s_down[:, 2 * c + 1],
            ]
        ).astype(np.float32)
        sgb = np.ascontiguousarray(
            np.broadcast_to(srow.astype(bf), (128, 6 * KT))
        )
        sgu = np.ascontiguousarray(
            np.broadcast_to(srow[: 4 * KT].astype(np.float32), (128, 4 * KT))
        )
        in_maps.append({"xp": xp, "wgu": wgu, "wdp": wdp, "sgb": sgb, "sgu": sgu})
    return in_maps


def kernel(x, w_gate, s_gate, w_up, s_up, w_down, s_down, _trace=False):
    x = np.asarray(x, np.float32)
    w_gate = np.asarray(w_gate, np.float32)
    w_up = np.asarray(w_up, np.float32)
    w_down = np.asarray(w_down, np.float32)
    s_gate = np.asarray(s_gate, np.float32)
    s_up = np.asarray(s_up, np.float32)
    s_down = np.asarray(s_down, np.float32)

    nc = _get_program()
    in_maps = _prep_inputs(x, w_gate, s_gate, w_up, s_up, w_down, s_down)
    res = bass_utils.run_bass_kernel_spmd(
        nc, in_maps, core_ids=list(range(NCORES)), trace=_trace
    )
    y = np.zeros((T, H), np.float32)
    for c in range(NCORES):
        y += res.results[c]["y"].astype(np.float32)
    if _trace:
        _CACHE["last_results"] = res
    return y


# revision 9
# speedup vs baseline: 1.3158x; 1.3158x over previous
"""DeepseekV3 FP8-block-dequant SwiGLU MLP on 8 TRN2 NeuronCores.

Computation: y = (silu(x @ dq(w_gate).T) * (x @ dq(w_up).T)) @ dq(w_down).T
with dq(w)[o,i] = w[o,i] * s[o//128, i//128].

Sharding: tensor-parallel over the F=2048 intermediate dim, 256 per core.
Each core computes a rank-256 partial of the output; partials are summed
on the host (the unshard step for a contraction-sharded output).

Device layout (prepared host-side, bf16):
  xp   [128, 56*512] : xp[p, k*512+t] = x[t, k*128+p]            (same on all cores)
  wgu  [128, 56*512] : wgu[p, k*512+m*128+f] = w_gate[c*256+m*128+f, k*128+p]; +256 up
  wdp  [128, 2*7168] : wdp[p, k2*7168+h] = w_down[h, c*256+k2*128+p]
  sgb  [128, 336]    : bf16 scale grid rows, 56 cols per block-row:
                       [gate m=0 | gate m=1 | up m=0 | up m=1 | down k2=0 | down k2=1]
  sgu  [128, 224]    : tiny fp32 grid, only used as PE-warmup matmul fodder
All matmuls contract over the partition dim. Dequant is elementwise
w *= scale done in-place in the weight landing buffers by wide DVE
tensor-tensor ops whose scale operand is a stride-0 broadcast AP over the
tiny sgb grid (each 128-column group of a k-tile shares one scale value),
so no full-size scale tensor is ever DMAed: per-core HBM traffic is
25.7 MB vs 36.7 MB with materialized scale broadcasts. At the ~340 GB/s
per-core DMA ceiling that is the difference between 108 us (DMA-bound)
and ~76 us (near the 71.7 us bf16 PE roofline).
"""

import sys

if "/opt/trn_rl_repo" not in sys.path:
    sys.path.insert(0, "/opt/trn_rl_repo")

from contextlib import ExitStack

import ml_dtypes
import numpy as np

import concourse.bacc as bacc
import concourse.mybir as mybir
import concourse.tile as tile
from concourse import bass_utils
from concourse.bass import AP as _AP

T, H, F = 512, 7168, 2048
NCORES = 8
FC = F // NCORES  # 256 intermediate channels per core
KT = H // 128  # 56 contraction k-tiles for gate/up
HN = H // 512  # 14 output column chunks for down matmul
BF16 = mybir.dt.bfloat16
F32 = mybir.dt.float32

_CACHE = {}


def _bview(ap, dims, off=0):
    """AP over `ap`'s tensor: keep the partition dim, replace the free dims
    with explicit (stride, size) pairs (stride 0 = broadcast), shift by off
    elements."""
    return _AP(ap.tensor, ap.offset + off, [list(ap.ap[0])] + [[s, n] for s, n in dims])


def _build_program(repeats=1, warmup=True):
    nc = bacc.Bacc("TRN2", target_bir_lowering=False, debug=False, num_devices=NCORES)

    xd = nc.dram_tensor("xp", [128, KT * T], BF16, kind="ExternalInput")
    wgud = nc.dram_tensor("wgu", [128, KT * 2 * FC], BF16, kind="ExternalInput")
    wdd = nc.dram_tensor("wdp", [128, 2 * H], BF16, kind="ExternalInput")
    sgbd = nc.dram_tensor("sgb", [128, 6 * KT], BF16, kind="ExternalInput")
    sgud = nc.dram_tensor("sgu", [128, 4 * KT], F32, kind="ExternalInput")
    yd = nc.dram_tensor("y", [T, H], BF16, kind="ExternalOutput")

    with tile.TileContext(nc) as tc, ExitStack() as ctx:
        consts = ctx.enter_context(tc.tile_pool(name="consts", bufs=1))
        xpool = ctx.enter_context(tc.tile_pool(name="xpool", bufs=3))
        wpool = ctx.enter_context(tc.tile_pool(name="wpool", bufs=3))
        silpool = ctx.enter_context(tc.tile_pool(name="silpool", bufs=2))
        hpool = ctx.enter_context(tc.tile_pool(name="hpool", bufs=2))
        wdraw_pool = ctx.enter_context(tc.tile_pool(name="wdraw", bufs=2))
        ystage = ctx.enter_context(tc.tile_pool(name="ystage", bufs=2))
        pgu = ctx.enter_context(tc.tile_pool(name="pgu", bufs=4, space="PSUM"))
        pd = ctx.enter_context(tc.tile_pool(name="pd", bufs=4, space="PSUM"))

        sgu_sb = consts.tile([128, 4 * KT], F32, name="sgu_sb", tag="sgu_sb")
        nc.sync.dma_start(sgu_sb[:], sgud.ap())
        sgb_sb = consts.tile([128, 6 * KT], BF16, name="sgb_sb", tag="sgb_sb")
        nc.sync.dma_start(sgb_sb[:], sgbd.ap())
        # dummy sigmoid at kernel start: loads the ACT sigmoid LUT while the
        # pipeline fills, so the real sigmoids at the phase-1->2 transition
        # don't pay the ~1.3us table-load on the critical path
        sig_warm = consts.tile([128, 1], BF16, name="sig_warm", tag="sig_warm")
        nc.scalar.activation(
            sig_warm[:], sgu_sb[:, :1], mybir.ActivationFunctionType.Sigmoid
        )

        # ---- PE warmup (once per program, NOT per repeat body): dummy fp32
        # matmuls on the (tiny, already-loaded) scale grid keep the PE HAM
        # activity window busy during the DMA pipeline fill, so the real
        # matmul stream starts at 2.4 GHz. In a repeat stream the PE stays
        # hot, so later bodies don't need (or pay for) the warmup.
        ps_warm = pd.tile([128, 512], F32, name="ps_warm", tag="pd")
        for _ in range(10 if warmup else 0):
            nc.tensor.matmul(
                ps_warm[:, : 2 * KT],
                sgu_sb[:, :128],
                sgu_sb[:, : 2 * KT],
                start=True,
                stop=True,
            )

        def dq_gu(wc, k0, j0, cs):
            """Dequant k-tiles [j0, j0+cs) of the chunk starting at k-tile k0:
            wc cols (k, group g of 4, 128 inner) *= sgb[g*KT + k0+k],
            broadcast over the 128 inner columns."""
            wv = _bview(wc[:], [(512, cs), (128, 4), (1, 128)], off=j0 * 512)
            sv = _bview(sgb_sb[:], [(1, cs), (KT, 4), (0, 128)], off=k0 + j0)
            nc.vector.tensor_mul(wv, wv, sv)

        def dq_d(wdr, k2, n):
            """Dequant down-weight cols [n*512,(n+1)*512) of the k2 tile:
            4 hb groups of 128 *= sgb[4*KT + k2*KT + 4n + hb]."""
            wv = _bview(wdr[:], [(128, 4), (1, 128)], off=n * 512)
            sv = _bview(sgb_sb[:], [(1, 4), (0, 128)], off=4 * KT + k2 * KT + 4 * n)
            nc.vector.tensor_mul(wv, wv, sv)

        def emit_body():
            # ---- phase 1: gT/uT = dq(w).T @ x.T tiles, accumulated over 56 k
            psg = [pgu.tile([128, T], F32, name=f"psg{m}", tag="p1") for m in range(2)]
            psu = [pgu.tile([128, T], F32, name=f"psu{m}", tag="p1") for m in range(2)]

            # moderate first chunk (fewer chunk boundaries beat tiny head
            # chunks); small last chunk keeps the post-DMA compute tail short
            chunks = [4, 8, 8, 8, 8, 8, 8, 4]
            assert sum(chunks) == KT

            k = 0
            for g, cs in enumerate(chunks):
                nb = {4: 2, 8: 3}[cs]
                cols = slice(k * T, (k + cs) * T)
                wc = wpool.tile(
                    [128, cs * 2 * FC], BF16, name=f"wc{g}", tag=f"wc{cs}", bufs=nb
                )
                nc.sync.dma_start(wc[:], wgud.ap()[:, cols])
                xc = xpool.tile(
                    [128, cs * T], BF16, name=f"xc{g}", tag=f"xc{cs}", bufs=nb
                )
                nc.sync.dma_start(xc[:], xd.ap()[:, cols])
                # in-place dequant of the chunk; the last (small) chunk is
                # dequantized in two halves so its matmuls are not gated on
                # one full-chunk DVE op right at the phase-1 tail
                if g == len(chunks) - 1 and cs >= 2:
                    dq_gu(wc, k, 0, cs // 2)
                    dq_gu(wc, k, cs // 2, cs - cs // 2)
                else:
                    dq_gu(wc, k, 0, cs)
                for j in range(cs):
                    start, stop = (k == 0), (k == KT - 1)
                    rhs = xc[:, j * T : (j + 1) * T]
                    for m in range(2):
                        nc.tensor.matmul(
                            psg[m][:],
                            wc[:, j * 512 + m * 128 : j * 512 + (m + 1) * 128],
                            rhs,
                            start=start,
                            stop=stop,
                        )
                        nc.tensor.matmul(
                            psu[m][:],
                            wc[:, j * 512 + 256 + m * 128 : j * 512 + (m + 1) * 128 + 256],
                            rhs,
                            start=start,
                            stop=stop,
                        )
                    k += 1

            # ---- down-proj weight loads on SP's queue after the gate/up
            # chunk reads. SP's per-body stream (18.4 MB, ~54 us) drains
            # faster than the body period, so with cross-body pipelining
            # these arrive well before phase 2 needs them (~48 us in); the
            # ACT queue carries only y writes and must NOT carry these (ACT
            # DMA issues are in-order with ACT compute, which would delay
            # them behind the previous body's PSUM-drain copies).
            wdr = [
                wdraw_pool.tile([128, H], BF16, name=f"wdr{i}", tag="wdr")
                for i in range(2)
            ]
            for half in range(2):
                lo, hi = half * (H // 2), (half + 1) * (H // 2)
                for k2 in range(2):
                    nc.sync.dma_start(
                        wdr[k2][:, lo:hi], wdd.ap()[:, k2 * H + lo : k2 * H + hi]
                    )

            # dequant only the FIRST phase-2 weight chunk before the h chain:
            # the first down-matmul needs h(t=0) and wdq(n=0) — both on DVE's
            # in-order queue — so anything more here just delays the h chain
            for k2 in range(2):
                dq_d(wdr[k2], k2, 0)

            # ---- h = silu(g) * u = sigmoid(g) * g * u, in [128, 128] column
            # slices so phase 2's t=0 matmuls can start early
            sil = [
                silpool.tile([128, T], BF16, name=f"sil{m}", tag="sil")
                for m in range(2)
            ]
            tmp = [
                silpool.tile([128, T], BF16, name=f"sgm{m}", tag="sgm")
                for m in range(2)
            ]
            hts = [
                hpool.tile([128, T], BF16, name=f"ht{m}", tag="ht") for m in range(2)
            ]
            for tt in range(4):
                sl = slice(tt * 128, (tt + 1) * 128)
                for m in range(2):
                    # sigmoid + two muls rather than Silu: CoreSim lacks Silu,
                    # and on HW the Silu LUT measured slower end-to-end
                    nc.scalar.activation(
                        sil[m][:, sl],
                        psg[m][:, sl],
                        mybir.ActivationFunctionType.Sigmoid,
                    )
                    nc.vector.tensor_mul(tmp[m][:, sl], sil[m][:, sl], psg[m][:, sl])
                    nc.vector.tensor_mul(hts[m][:, sl], tmp[m][:, sl], psu[m][:, sl])

            # ---- remaining phase 2 in-place dequant (n-major so early n
            # chunks are ready first), then y_partial[t, h] = hT.T @ dq(wd)
            for n in range(1, HN):
                for k2 in range(2):
                    dq_d(wdr[k2], k2, n)

            for t in range(4):
                for half in range(2):
                    ystg = ystage.tile([128, H // 2], BF16, name=f"ys{t}{half}", tag="ys")
                    for nh in range(HN // 2):
                        n = half * (HN // 2) + nh
                        ps = pd.tile([128, 512], F32, name=f"ps{t}_{n}", tag="pd")
                        for k2 in range(2):
                            nc.tensor.matmul(
                                ps[:],
                                hts[k2][:, t * 128 : (t + 1) * 128],
                                wdr[k2][:, n * 512 : (n + 1) * 512],
                                start=(k2 == 0),
                                stop=(k2 == 1),
                            )
                        dst = ystg[:, nh * 512 : (nh + 1) * 512]
                        if (n + t) % 4 == 0:
                            nc.vector.tensor_copy(dst, ps[:])
                        else:
                            nc.scalar.copy(dst, ps[:])
                        # split the very last output transfer so the DMA tail
                        # after the final copy is short
                        if t == 3 and half == 1 and nh == 3:
                            nc.sync.dma_start(
                                yd.ap()[
                                    t * 128 : (t + 1) * 128,
                                    H // 2 : H // 2 + 4 * 512,
                                ],
                                ystg[:, : 4 * 512],
                            )
                    lo = 4 * 512 if (t == 3 and half == 1) else 0
                    nc.sync.dma_start(
                        yd.ap()[
                            t * 128 : (t + 1) * 128,
                            half * (H // 2) + lo : (half + 1) * (H // 2),
                        ],
                        ystg[:, lo:],
                    )

        for _rep in range(repeats):
            emit_body()

    nc.compile()
    return nc


def _get_program():
    if "nc" not in _CACHE:
        _CACHE["nc"] = _build_program()
    return _CACHE["nc"]


def _prep_inputs(x, w_gate, s_gate, w_up, s_up, w_down, s_down):
    bf = ml_dtypes.bfloat16
    # x -> [p, k, t] -> [128, KT*T]
    xp = np.ascontiguousarray(
        x.reshape(T, KT, 128).transpose(2, 1, 0).reshape(128, KT * T)
    ).astype(bf)
    in_maps = []
    for c in range(NCORES):
        gsl = slice(c * FC, (c + 1) * FC)
        ag = w_gate[gsl].reshape(FC, KT, 128).transpose(2, 1, 0)  # [p, k, f]
        au = w_up[gsl].reshape(FC, KT, 128).transpose(2, 1, 0)
        wgu = np.ascontiguousarray(
            np.concatenate([ag, au], axis=2).reshape(128, KT * 2 * FC)
        ).astype(bf)
        wdp = np.ascontiguousarray(
            w_down[:, gsl].reshape(H, 2, 128).transpose(2, 1, 0).reshape(128, 2 * H)
        ).astype(bf)
        # scale grid rows (bf16, replication only): 56 cols per block-row of
        # this core's weights, in the order the broadcast-AP dequant reads:
        # [gate m=0 | gate m=1 | up m=0 | up m=1 | down k2=0 | down k2=1]
        srow = np.concatenate(
            [
                s_gate[2 * c],
                s_gate[2 * c + 1],
                s_up[2 * c],
                s_up[2 * c + 1],
                s_down[:, 2 * c],
                s_down[:, 2 * c + 1],
            ]
        ).astype(np.float32)
        sgb = np.ascontiguousarray(
            np.broadcast_to(srow.astype(bf), (128, 6 * KT))
        )
        sgu = np.ascontiguousarray(
            np.broadcast_to(srow[: 4 * KT].astype(np.float32), (128, 4 * KT))
        )
        in_maps.append({"xp": xp, "wgu": wgu, "wdp": wdp, "sgb": sgb, "sgu": sgu})
    return in_maps


def kernel(x, w_gate, s_gate, w_up, s_up, w_down, s_down, _trace=False):
    x = np.asarray(x, np.float32)
    w_gate = np.asarray(w_gate, np.float32)
    w_up = np.asarray(w_up, np.float32)
    w_down = np.asarray(w_down, np.float32)
    s_gate = np.asarray(s_gate, np.float32)
    s_up = np.asarray(s_up, np.float32)
    s_down = np.asarray(s_down, np.float32)

    nc = _get_program()
    in_maps = _prep_inputs(x, w_gate, s_gate, w_up, s_up, w_down, s_down)
    res = bass_utils.run_bass_kernel_spmd(
        nc, in_maps, core_ids=list(range(NCORES)), trace=_trace
    )
    y = np.zeros((T, H), np.float32)
    for c in range(NCORES):
        y += res.results[c]["y"].astype(np.float32)
    if _trace:
        _CACHE["last_results"] = res
    return y
